# revision 1
# baseline (speedup 1.0000x reference)
"""BitLinear (BitNet b1.58-style) kernel for Trainium2, 8-core SPMD.

Reference computation (fp32):
    scale_w = max(mean(|W|), EPS)                       # scalar over all of W
    dq_w    = clip(round(W / scale_w), -1, 1) * scale_w # ternary weight
    amax_t  = max(max_j |x[t, j]|, EPS)                 # per token
    s_t     = 127 / amax_t
    dq_x    = round(x * s_t) / s_t                      # 8-bit absmax act quant
    out     = dq_x @ dq_w.T + b

Device strategy (data-parallel over tokens):
  * x is flattened to [8192, 4096] tokens and sharded 8 ways (1024/core).
  * W is transposed on the host once (layout choice) so each core can
    stream W.T = [in, out] tiles with K on partitions, and replicated.
  * The |W| mean reduction is sharded 8 ways (each core reduces 512 rows
    of W) and combined with a 4-byte AllReduce.
  * The matmul runs in bf16 on the PE array: q_x in [-127,127] and
    q_w in {-1,0,1} are bf16-exact, and fp32 PSUM accumulation of 4096
    products bounded by 127 is exact (< 2^24). The scales are folded
    into a per-token multiplier applied on PSUM eviction:
        out[t, :] = (q_x @ q_w.T)[t, :] * (scale_w * amax_t / 127) + b
  * Rounding uses the fp32 magic-constant trick (v + 1.5*2^23) - 1.5*2^23,
    round-to-nearest-even, matching jnp.round.  For W the multiply by
    1/scale_w and the +C rounding are SEPARATE instructions (ACT then
    DVE) so the double rounding matches the reference's fl(W/scale)
    then round() exactly; a fused FMA would single-round and flip a few
    ternary weights at the 0.5/1.5 boundaries.
"""

import numpy as np

from concourse import bacc, bass_isa, masks, mybir, tile
from concourse.tile import add_dep_helper
from concourse.bass_utils import run_bass_kernel_spmd

F32 = mybir.dt.float32
BF16 = mybir.dt.bfloat16
AX = mybir.AxisListType
OP = mybir.AluOpType
AF = mybir.ActivationFunctionType

EPS = 1e-6
QMAX = 127.0
C_MAGIC = 1.5 * 2.0**23  # fp32 RNE rounding constant

N_CORES = 8
B, S, D_IN, D_OUT = 4, 2048, 4096, 4096
T_FULL = B * S  # 8192 tokens
T = T_FULL // N_CORES  # 1024 tokens per core
SR = D_OUT // N_CORES  # 512 W rows per core for the |W| mean


def build_bass(t=T, di=D_IN, do=D_OUT, sr=SR, n_cores=N_CORES):
    """Emit the per-core SPMD program. All cores run the same code on
    their own shard; the only cross-core op is a 4-byte AllReduce."""
    assert t % 128 == 0 and di % 512 == 0 and do % 512 == 0 and sr % 128 == 0
    mt = t // 128  # token tiles
    kt = di // 128  # contraction tiles
    nt = do // 512  # output-column blocks
    assert mt <= 8, "one PSUM bank per token tile"

    nc = bacc.Bacc(None)
    xs_d = nc.dram_tensor("xs", [t, di], F32, kind="ExternalInput")
    wt_d = nc.dram_tensor("wt", [di, do], F32, kind="ExternalInput")
    ws_d = nc.dram_tensor("wshard", [sr, di], F32, kind="ExternalInput")
    b_d = nc.dram_tensor("bias", [1, do], F32, kind="ExternalInput")
    out_d = nc.dram_tensor("out", [t, do], F32, kind="ExternalOutput")
    probe_d = nc.dram_tensor("probe", [1, 8], F32, kind="ExternalOutput")

    with tile.TileContext(nc) as tc:
        with (
            tc.tile_pool(name="persist", bufs=1) as persist,
            tc.tile_pool(name="small", bufs=2) as small,
            tc.tile_pool(name="dram", bufs=1, space="DRAM") as dram,
        ):
            # ---- constants -------------------------------------------------
            ident = persist.tile([128, 128], BF16)
            masks.make_identity(nc, ident[:])

            # probe: ACT Copy/Identity bias exactness at C magnitude
            prb_in = small.tile([1, 4], F32)
            nc.vector.memset(prb_in[:], 0.37)
            prb_bias = small.tile([1, 1], F32)
            nc.vector.memset(prb_bias[:], C_MAGIC)
            prb_out = small.tile([1, 8], F32)
            nc.scalar.activation(prb_out[:, 0:4], prb_in[:], AF.Copy, bias=C_MAGIC)
            nc.scalar.activation(
                prb_out[:, 4:8], prb_in[:], AF.Identity, bias=prb_bias[:, 0:1]
            )
            nc.sync.dma_start(probe_d[:], prb_out[:])

            # bias broadcast: load into row 0 of bb, broadcast in place
            bb = persist.tile([128, do], F32)
            nc.sync.dma_start(bb[0:1, :], b_d[:])
            nc.gpsimd.partition_broadcast(bb[:], bb[0:1, :], channels=128)

            # ---- phase A pools + sharded mean(|W|) -------------------------
            qxT = persist.tile([128, kt, t], BF16)
            s_all = persist.tile([128, mt], F32)  # 127/amax per token
            c_all = persist.tile([128, mt], F32)  # scale_w*amax/127 per token
            scw = persist.tile([128, 1], F32)
            inv_w = persist.tile([128, 1], F32)
            with (
                tc.tile_pool(name="xphase", bufs=3) as xphase,
                tc.tile_pool(name="psumA", bufs=4, space="PSUM") as psumA,
            ):
                # mean(|W|) shard: full-row tiles (the f32 summation order is
                # deliberately row-sequential -- it lands scale_w within an
                # ulp of the jax reference's mean, avoiding ternary-weight
                # flips at round boundaries), abs-sum on DVE, then a 4-byte
                # AllReduce.  Own tile tag so the x pipeline starts at t=0.
                wsum_p = small.tile([128, sr // 128], F32)
                for i in range(sr // 128):
                    wti = xphase.tile([128, di], F32, tag="wmean", bufs=2, name="wti")
                    eng = nc.sync if i % 2 == 0 else nc.scalar
                    eng.dma_start(wti[:], ws_d[i * 128 : (i + 1) * 128, :])
                    nc.vector.tensor_reduce(
                        out=wsum_p[:, i : i + 1],
                        in_=wti[:],
                        axis=AX.X,
                        op=OP.add,
                        apply_absolute_value=True,
                    )
                wsum1 = small.tile([128, 1], F32)
                nc.vector.tensor_reduce(
                    out=wsum1[:], in_=wsum_p[:], axis=AX.X, op=OP.add
                )
                wsum_all = small.tile([128, 1], F32)
                nc.gpsimd.partition_all_reduce(
                    wsum_all[:], wsum1[:], channels=128,
                    reduce_op=bass_isa.ReduceOp.add,
                )
                cc_in = dram.tile([1, 1], F32)
                cc_out = dram.tile([1, 1], F32, addr_space="Shared")
                nc.sync.dma_start(cc_in[:], wsum_all[0:1, 0:1])
                nc.gpsimd.collective_compute(
                    "AllReduce",
                    OP.add,
                    replica_groups=[list(range(n_cores))],
                    ins=[cc_in[:]],
                    outs=[cc_out[:]],
                )
                tot = small.tile([1, 1], F32)
                nc.sync.dma_start(tot[:], cc_out[:])
                tot_b = small.tile([128, 1], F32)
                nc.gpsimd.partition_broadcast(tot_b[:], tot[:], channels=128)
                # scale_w = max(total / (D_IN*D_OUT), EPS); inv_w = 1/scale_w
                nc.vector.tensor_scalar(
                    scw[:], tot_b[:], 1.0 / (di * do), EPS, op0=OP.mult, op1=OP.max
                )
                nc.vector.reciprocal(inv_w[:], scw[:])

                # ---- phase A: activation quant + transpose ------------------
                for m in range(mt):
                    xtl = xphase.tile([128, di], F32, tag="x_in")
                    eng = nc.sync if m % 2 == 0 else nc.scalar
                    eng.dma_start(xtl[:], xs_d[m * 128 : (m + 1) * 128, :])
                    amax = xphase.tile([128, 1], F32, tag="amax")
                    nc.vector.tensor_reduce(
                        out=amax[:],
                        in_=xtl[:],
                        axis=AX.X,
                        op=OP.max,
                        apply_absolute_value=True,
                    )
                    amax_c = xphase.tile([128, 1], F32, tag="amax_c")
                    nc.vector.tensor_scalar(amax_c[:], amax[:], EPS, None, op0=OP.max)
                    rec = xphase.tile([128, 1], F32, tag="rec")
                    nc.vector.reciprocal(rec[:], amax_c[:])
                    nc.vector.tensor_scalar(
                        s_all[:, m : m + 1], rec[:], QMAX, None, op0=OP.mult
                    )
                    nc.vector.tensor_scalar(
                        c_all[:, m : m + 1],
                        amax_c[:],
                        scw[:, 0:1],
                        1.0 / QMAX,
                        op0=OP.mult,
                        op1=OP.mult,
                    )
                    # q_x = round(x*s): affine+round on ACT in place
                    # (probe-verified exact), then subtract C on DVE -> bf16
                    last_round = nc.scalar.activation(
                        xtl[:], xtl[:], AF.Copy,
                        bias=C_MAGIC, scale=s_all[:, m : m + 1],
                    )
                    qx = xphase.tile([128, di], BF16, tag="qx", bufs=2)
                    last_qxsub = nc.vector.tensor_scalar(
                        qx[:], xtl[:], C_MAGIC, None, op0=OP.subtract
                    )
                    # transpose 128x128 blocks via PE into resident qxT
                    for j in range(kt):
                        pt = psumA.tile([128, 128], BF16, tag="pt")
                        last_tp = nc.tensor.transpose(
                            pt[:], qx[:, j * 128 : (j + 1) * 128], ident[:]
                        )
                        dst = qxT[:, j, m * 128 : (m + 1) * 128]
                        if j % 2 == 0:
                            nc.vector.tensor_copy(dst, pt[:])
                        else:
                            nc.scalar.copy(dst, pt[:])

            # ---- phase B: stream W, quantize, matmul, scale, store ---------
            # loop order n -> k -> m: each quantized W tile feeds all mt
            # token tiles back to back; W is read from HBM exactly once.
            with (
                tc.tile_pool(name="wpipe", bufs=6) as wpipe,
                tc.tile_pool(name="opipe", bufs=3) as opipe,
                tc.tile_pool(name="psumB", bufs=1, space="PSUM") as psumB,
            ):
                for n in range(nt):
                    # allocate high-m first: psum banks that overlap the
                    # (release-gated) phase-A transpose banks then belong to
                    # the LAST token tiles, whose own transposes finish last
                    # anyway -- early token tiles start unhindered.
                    pss = [None] * mt
                    for m in reversed(range(mt)):
                        pss[m] = psumB.tile([128, 512], F32, tag=f"mm{m}", name="ps")
                    for k in range(kt):
                        wtl = wpipe.tile([128, 512], F32, tag="w_in")
                        nc.gpsimd.dma_start(
                            wtl[:],
                            wt_d[k * 128 : (k + 1) * 128, n * 512 : (n + 1) * 512],
                        )
                        # u = W*inv_w on ACT (separate rounding step matches
                        # the reference's fl(W/scale)); then +C round and clip
                        # in the C-offset domain on DVE; sub C -> bf16
                        wa = nc.scalar.activation(
                            wtl[:], wtl[:], AF.Copy, bias=0.0, scale=inv_w[:, 0:1]
                        )
                        wc = nc.vector.tensor_scalar(
                            wtl[:], wtl[:], C_MAGIC, C_MAGIC + 1.0,
                            op0=OP.add, op1=OP.min,
                        )
                        if n == 0 and k == 0:
                            add_dep_helper(
                                wa.ins, last_round.ins, sync=False,
                                reason="phase-A ACT work before W affine",
                            )
                            add_dep_helper(
                                wc.ins, last_qxsub.ins, sync=False,
                                reason="phase-A DVE work before W clip",
                            )
                        qw = wpipe.tile([128, 512], BF16, tag="w_q")
                        nc.vector.tensor_scalar(
                            qw[:], wtl[:], C_MAGIC - 1.0, C_MAGIC,
                            op0=OP.max, op1=OP.subtract,
                        )
                        for m in range(mt):
                            mm = nc.tensor.matmul(
                                pss[m][:],
                                qxT[:, k, m * 128 : (m + 1) * 128],
                                qw[:],
                                start=(k == 0),
                                stop=(k == kt - 1),
                            )
                            if n == 0 and k == 0:
                                add_dep_helper(
                                    mm.ins, last_tp.ins, sync=False,
                                    reason="order all transposes before matmuls",
                                )
                    for m in range(mt):
                        ot = opipe.tile([128, 512], F32, tag="o_scaled")
                        nc.scalar.activation(
                            ot[:], pss[m][:], AF.Copy,
                            bias=0.0, scale=c_all[:, m : m + 1],
                        )
                        ot2 = opipe.tile([128, 512], F32, tag="o_final")
                        nc.gpsimd.tensor_tensor(
                            ot2[:], ot[:], bb[:, n * 512 : (n + 1) * 512], op=OP.add
                        )
                        nc.sync.dma_start(
                            out_d[m * 128 : (m + 1) * 128, n * 512 : (n + 1) * 512],
                            ot2[:],
                        )
    nc.compile()
    return nc


_PROGRAM = None


def _get_program():
    global _PROGRAM
    if _PROGRAM is None:
        _PROGRAM = build_bass()
    return _PROGRAM


def make_in_maps(x, W, b):
    """Shard full inputs into the 8 per-core input dicts."""
    x = np.ascontiguousarray(x, dtype=np.float32).reshape(T_FULL, D_IN)
    W = np.ascontiguousarray(W, dtype=np.float32)
    b = np.ascontiguousarray(b, dtype=np.float32).reshape(1, D_OUT)
    wt = np.ascontiguousarray(W.T)  # [in, out]
    in_maps = []
    for c in range(N_CORES):
        in_maps.append(
            {
                "xs": x[c * T : (c + 1) * T],
                "wt": wt,
                "wshard": np.ascontiguousarray(W[c * SR : (c + 1) * SR]),
                "bias": b,
            }
        )
    return in_maps


def kernel(x, W, b, trace=False, tmpdir=None):
    nc = _get_program()
    res = run_bass_kernel_spmd(
        nc,
        make_in_maps(x, W, b),
        core_ids=list(range(N_CORES)),
        trace=trace,
        tmpdir=tmpdir,
    )
    out = np.concatenate([res.results[c]["out"] for c in range(N_CORES)], axis=0)
    out = out.reshape(B, S, D_OUT)
    if trace:
        kernel.last_results = res
    return out



# revision 6
# speedup vs baseline: 1.2033x; 1.2033x over previous
"""BitLinear (BitNet b1.58-style) kernel for Trainium2, 8-core SPMD.

Reference computation (fp32):
    scale_w = max(mean(|W|), EPS)                       # scalar over all of W
    dq_w    = clip(round(W / scale_w), -1, 1) * scale_w # ternary weight
    amax_t  = max(max_j |x[t, j]|, EPS)                 # per token
    s_t     = 127 / amax_t
    dq_x    = round(x * s_t) / s_t                      # 8-bit absmax act quant
    out     = dq_x @ dq_w.T + b

Device strategy (data-parallel over tokens, fp8 DoubleRow matmul):
  * x is flattened to [8192, 4096] tokens, sharded 8 ways (1024/core) and
    shipped as bf16 (the 2e-2 tolerance dwarfs bf16 rounding; verified by
    exact simulation of this kernel's arithmetic: rel-err 0.0155).
  * q_x = round(x*s) in [-127,127].  The first KF8=24 k-tiles are stored
    as fp8e4 (e4m3 RNE - the only lossy step) and their matmuls run pairs
    of k-tiles with perf_mode=DoubleRow (2 fp8 weights/PE cell, K=256 per
    instruction) at ~1.44x bf16 throughput.  The last 8 k-tiles stay
    exact bf16 to buy error margin (the q ints <= 128 are bf16-exact).
    q_w in {-1,0,1} is exact in both fp8e4 and bf16.
  * x is quantized token-major (per-token scale = per-partition ACT scale,
    fused multiply+round via the fp32 magic constant C), then moved to
    feature-major by xbar DMA-transpose (2-byte dtype) + a DVE cast copy.
    The PE runs no transposes at all.
  * W ships as W.T, column-blocked in per-core rotated order, so each
    core's first block doubles as its 1/8 shard for the |W| mean; the
    4-byte AllReduce hides under the x DMA + quant.
  * Eviction: psum * c_t on DVE (per-partition scalar), bias on GPSIMD,
    stores on the ACT HWDGE ring so the sync ring only carries loads.
"""

import numpy as np
import ml_dtypes

from concourse import bacc, bass_isa, mybir, tile
from concourse.bass_utils import run_bass_kernel_spmd

F32 = mybir.dt.float32
BF16 = mybir.dt.bfloat16
FP8 = mybir.dt.float8e4
AX = mybir.AxisListType
OP = mybir.AluOpType
AF = mybir.ActivationFunctionType
PM = mybir.MatmulPerfMode

EPS = 1e-6
QMAX = 127.0
C_MAGIC = 1.5 * 2.0**23  # fp32 RNE rounding constant

N_CORES = 8
B, S, D_IN, D_OUT = 4, 2048, 4096, 4096
T_FULL = B * S
T = T_FULL // N_CORES  # 1024 tokens per core
KT = D_IN // 128  # 32 contraction k-tiles
KF8 = 24  # k-tiles carried in fp8 (12 DoubleRow pairs)
KBF = KT - KF8  # k-tiles carried in bf16 (exact)
NT = D_OUT // 512  # 8 output column blocks
MT = T // 128  # 8 token tiles


def build_bass():
    nc = bacc.Bacc(None)
    xs_d = nc.dram_tensor("xs", [T, D_IN], BF16, kind="ExternalInput")
    wtb_d = nc.dram_tensor("wtb", [NT * D_IN, 512], F32, kind="ExternalInput")
    b_d = nc.dram_tensor("bias", [1, D_OUT], F32, kind="ExternalInput")
    out_d = nc.dram_tensor("out", [T, D_OUT], F32, kind="ExternalOutput")

    def wslice(j, qi):
        # quarter qi of column-block j: [1024, 512] of wtb, k-major
        r0 = (j * 4 + qi) * 1024
        return wtb_d[r0 : r0 + 1024, :].rearrange("(a p) c -> p a c", p=128)

    with tile.TileContext(nc) as tc:
        with (
            tc.tile_pool(name="persist", bufs=1) as persist,
            tc.tile_pool(name="small", bufs=2) as small,
            tc.tile_pool(name="wpipe", bufs=2) as wpipe,
            tc.tile_pool(name="qwpipe", bufs=1) as qwpipe,
            tc.tile_pool(name="dram", bufs=1, space="DRAM") as dram,
        ):
            # persistent state
            qxT8 = persist.tile([128, KF8, T], FP8)
            qxTb = persist.tile([128, KBF, T], BF16)
            bb = persist.tile([128, D_OUT], F32)
            amax_e = persist.tile([128, MT], F32)  # max(amax, EPS) per token
            s_all = persist.tile([128, MT], F32)  # 127/amax per token
            c_all = persist.tile([128, MT], F32)  # amax*scale_w/127 per token
            scw = persist.tile([128, 1], F32)
            inv_w = persist.tile([128, 1], F32)

            # block-0 W quarters 0/1 prefetch on the ACT HWDGE ring (idle
            # until the phase-A transposes, which only become ready later)
            w0q = []
            for qi in range(2):
                wq = wpipe.tile([128, 8, 512], F32, tag="w_in", name="wq")
                nc.scalar.dma_start(wq[:], wslice(0, qi))
                w0q.append(wq)

            # bias: load row 0, broadcast across partitions (gpsimd)
            nc.sync.dma_start(bb[0:1, :], b_d[:])
            nc.gpsimd.partition_broadcast(bb[:], bb[0:1, :], channels=128)

            # ---- |W| mean over this core's first column block ------------
            wsum_p = small.tile([128, 8], F32)
            with tc.tile_pool(name="meanpipe", bufs=2) as meanpipe:
                for e in range(8):
                    mtl = meanpipe.tile([128, 4, 512], F32, tag="m_in")
                    nc.sync.dma_start(
                        mtl[:],
                        wtb_d[e * 512 : (e + 1) * 512, :].rearrange(
                            "(a p) c -> p a c", p=128
                        ),
                    )
                    nc.vector.tensor_reduce(
                        out=wsum_p[:, e : e + 1],
                        in_=mtl[:],
                        axis=AX.XY,
                        op=OP.add,
                        apply_absolute_value=True,
                    )
            wsum1 = small.tile([128, 1], F32)
            nc.vector.tensor_reduce(out=wsum1[:], in_=wsum_p[:], axis=AX.X, op=OP.add)
            wsum_all = small.tile([128, 1], F32)
            nc.gpsimd.partition_all_reduce(
                wsum_all[:], wsum1[:], channels=128, reduce_op=bass_isa.ReduceOp.add
            )
            cc_in = dram.tile([1, 1], F32)
            cc_out = dram.tile([1, 1], F32, addr_space="Shared")
            nc.gpsimd.dma_start(cc_in[:], wsum_all[0:1, 0:1])
            nc.gpsimd.collective_compute(
                "AllReduce",
                OP.add,
                replica_groups=[list(range(N_CORES))],
                ins=[cc_in[:]],
                outs=[cc_out[:]],
            )
            tot = small.tile([1, 1], F32)
            nc.gpsimd.dma_start(tot[:], cc_out[:])
            tot_b = small.tile([128, 1], F32)
            nc.gpsimd.partition_broadcast(tot_b[:], tot[:], channels=128)

            # ---- phase A: x quant (token-major) + xbar transpose ---------
            with (
                tc.tile_pool(name="xpipe", bufs=2) as xpipe,
                tc.tile_pool(name="tpipe", bufs=2) as tpipe,
            ):
                for m in range(MT):
                    xtl = xpipe.tile([128, D_IN], BF16, tag="x_in")
                    nc.sync.dma_start(xtl[:], xs_d[m * 128 : (m + 1) * 128, :])
                    amax = xpipe.tile([128, 1], F32, tag="amax")
                    nc.vector.tensor_reduce(
                        out=amax[:],
                        in_=xtl[:],
                        axis=AX.X,
                        op=OP.max,
                        apply_absolute_value=True,
                    )
                    nc.vector.tensor_scalar(
                        amax_e[:, m : m + 1], amax[:], EPS, None, op0=OP.max
                    )
                    rec = xpipe.tile([128, 1], F32, tag="rec")
                    nc.vector.reciprocal(rec[:], amax_e[:, m : m + 1])
                    nc.vector.tensor_scalar(
                        s_all[:, m : m + 1], rec[:], QMAX, None, op0=OP.mult
                    )
                    # q = round(x*s) on ACT: v = x*s + C (RNE integer),
                    # then -C -> bf16 on DVE (q ints are bf16-exact)
                    qxc = xpipe.tile([128, D_IN], BF16, tag="qxc")
                    for h in range(2):
                        hs = slice(h * 2048, (h + 1) * 2048)
                        qq = xpipe.tile([128, 2048], F32, tag="qq", bufs=1)
                        nc.scalar.activation(
                            qq[:], xtl[:, hs], AF.Copy,
                            bias=C_MAGIC, scale=s_all[:, m : m + 1],
                        )
                        nc.vector.tensor_scalar(
                            qxc[:, hs], qq[:], C_MAGIC, None, op0=OP.subtract
                        )
                    # feature-major via xbar transpose, then cast copy
                    t8 = tpipe.tile([128, KF8, 128], BF16, tag="t8")
                    nc.scalar.dma_start_transpose(t8[:], qxc[:, 0 : KF8 * 128])
                    nc.vector.tensor_copy(qxT8[:, :, m * 128 : (m + 1) * 128], t8[:])
                    tb = tpipe.tile([128, KBF, 128], BF16, tag="tb", bufs=1)
                    nc.scalar.dma_start_transpose(tb[:], qxc[:, KF8 * 128 :])
                    nc.vector.tensor_copy(qxTb[:, :, m * 128 : (m + 1) * 128], tb[:])

            # scale_w = max(total/(D_IN*D_OUT), EPS); inv_w = 1/scale_w.
            # Emitted after the x-quant DVE ops so the AllReduce wait does
            # not head-of-line-block the DVE queue.
            nc.vector.tensor_scalar(
                scw[:], tot_b[:], 1.0 / (D_IN * D_OUT), EPS, op0=OP.mult, op1=OP.max
            )
            nc.vector.reciprocal(inv_w[:], scw[:])
            nc.vector.tensor_scalar(
                c_all[:], amax_e[:], scw[:, 0:1], 1.0 / QMAX, op0=OP.mult, op1=OP.mult
            )

            # ---- phase B: quantize W, matmul, evict ----------------------
            # Software pipelined: W-quant for block j+1 is emitted before
            # the matmuls of block j, so its ACT/DVE work runs under them
            # and the evictions sit at the DVE queue tail when they finish.
            qw8 = {}  # (j, kp) -> fp8 pair tile, kp in [0, 12)
            qwb = {}  # (j, kq) -> bf16 pair tile, kq in [0, 4)

            def emit_wq(j, dma_tiles=()):
                for qi in range(4):
                    if qi < len(dma_tiles):
                        wq = dma_tiles[qi]
                    else:
                        wq = wpipe.tile([128, 8, 512], F32, tag="w_in", name="wq")
                        nc.sync.dma_start(wq[:], wslice(j, qi))
                    for pp in range(4):  # k-tile pairs within the quarter
                        kp = qi * 4 + pp
                        pr = wq[:, 2 * pp : 2 * pp + 2, :]
                        # u = W*inv_w on ACT (separate op: the follow-up
                        # +C round then sees fl(W/scale) like the ref)
                        nc.scalar.activation(
                            pr, pr, AF.Copy, bias=0.0, scale=inv_w[:, 0:1]
                        )
                        # +C RNE round and clip in the C domain
                        nc.vector.tensor_scalar(
                            pr, pr, C_MAGIC, C_MAGIC + 1.0, op0=OP.add, op1=OP.min
                        )
                        if kp < KF8 // 2:
                            qt = qwpipe.tile(
                                [128, 2, 512], FP8, tag=f"q8_{j % 2}", bufs=12
                            )
                            qw8[(j, kp)] = qt
                        else:
                            qt = qwpipe.tile(
                                [128, 2, 512], BF16, tag=f"qb_{j % 2}", bufs=4
                            )
                            qwb[(j, kp - KF8 // 2)] = qt
                        nc.vector.tensor_scalar(
                            qt[:], pr, C_MAGIC - 1.0, C_MAGIC,
                            op0=OP.max, op1=OP.subtract,
                        )

            def emit_mms(j, opipe, psumB):
                pss = [None] * MT
                for m in reversed(range(MT)):
                    pss[m] = psumB.tile([128, 512], F32, tag=f"mm{m}", name="ps")
                for kp in range(KF8 // 2):
                    for m in range(MT):
                        nc.tensor.matmul(
                            pss[m][:],
                            qxT8[:, 2 * kp : 2 * kp + 2, m * 128 : (m + 1) * 128],
                            qw8[(j, kp)][:],
                            start=(kp == 0),
                            stop=False,
                            perf_mode=PM.DoubleRow,
                        )
                    del qw8[(j, kp)]
                for kb in range(KBF):
                    for m in range(MT):
                        nc.tensor.matmul(
                            pss[m][:],
                            qxTb[:, kb, m * 128 : (m + 1) * 128],
                            qwb[(j, kb // 2)][:, kb % 2, :],
                            start=False,
                            stop=(kb == KBF - 1),
                        )
                    if kb % 2 == 1:
                        del qwb[(j, kb // 2)]
                for m in range(MT):
                    ot = opipe.tile([128, 512], F32, tag="o_scaled")
                    nc.vector.tensor_scalar(
                        ot[:], pss[m][:], c_all[:, m : m + 1], None, op0=OP.mult
                    )
                    ot2 = opipe.tile([128, 512], F32, tag="o_final")
                    nc.gpsimd.tensor_tensor(
                        ot2[:], ot[:], bb[:, j * 512 : (j + 1) * 512], op=OP.add
                    )
                    nc.scalar.dma_start(
                        out_d[m * 128 : (m + 1) * 128, j * 512 : (j + 1) * 512],
                        ot2[:],
                    )

            with (
                tc.tile_pool(name="opipe", bufs=3) as opipe,
                tc.tile_pool(name="psumB", bufs=1, space="PSUM") as psumB,
            ):
                emit_wq(0, dma_tiles=w0q)
                for j in range(1, NT):
                    emit_wq(j)
                    emit_mms(j - 1, opipe, psumB)
                emit_mms(NT - 1, opipe, psumB)

    nc.compile()
    return nc


_PROGRAM = None


def _get_program():
    global _PROGRAM
    if _PROGRAM is None:
        _PROGRAM = build_bass()
    return _PROGRAM


def make_in_maps(x, W, b):
    """Shard full inputs into the 8 per-core input dicts."""
    x = np.ascontiguousarray(x, dtype=np.float32).reshape(T_FULL, D_IN)
    xb = x.astype(ml_dtypes.bfloat16)
    W = np.ascontiguousarray(W, dtype=np.float32)
    b = np.ascontiguousarray(b, dtype=np.float32).reshape(1, D_OUT)
    wt = np.ascontiguousarray(W.T)  # [in, out]
    in_maps = []
    for c in range(N_CORES):
        blks = [(c + j) % N_CORES for j in range(NT)]
        wtb = np.concatenate(
            [wt[:, blk * 512 : (blk + 1) * 512] for blk in blks], axis=0
        )
        brot = np.concatenate(
            [b[:, blk * 512 : (blk + 1) * 512] for blk in blks], axis=1
        )
        in_maps.append(
            {
                "xs": np.ascontiguousarray(xb[c * T : (c + 1) * T]),
                "wtb": np.ascontiguousarray(wtb),
                "bias": np.ascontiguousarray(brot),
            }
        )
    return in_maps


def kernel(x, W, b, trace=False, tmpdir=None):
    nc = _get_program()
    res = run_bass_kernel_spmd(
        nc,
        make_in_maps(x, W, b),
        core_ids=list(range(N_CORES)),
        trace=trace,
        tmpdir=tmpdir,
    )
    out = np.empty((T_FULL, D_OUT), dtype=np.float32)
    for c in range(N_CORES):
        oc = res.results[c]["out"].reshape(T, NT, 512)
        for j in range(NT):
            blk = (c + j) % N_CORES
            out[c * T : (c + 1) * T, blk * 512 : (blk + 1) * 512] = oc[:, j]
    out = out.reshape(B, S, D_OUT)
    if trace:
        kernel.last_results = res
    return out
